# revision 15
# baseline (speedup 1.0000x reference)
"""DMPNN layer (message passing + scatter-add + GRU) on 8 Trainium2 cores.

Strategy: sort edges by destination on the host, partition destinations into
8 contiguous ranges (one per core). Each core computes messages for the edges
that TARGET its destination range, scatter-adds them locally into its
destination rows (no cross-core traffic), then runs the GRU on its row range.

On-device scatter-add: edges are packed (host-side) into 128-edge tiles, each
tile constrained to a 256-destination window [128*a_k, 128*a_k+256). The
window schedule {a_k} is shared by all 8 cores (SPMD: one program) and is
computed from the union of all cores' needs; slack is absorbed by zero-message
pad edges. Scatter = one-hot matmuls on the TensorEngine:
    aggT[:, win] += msg_chunk^T @ onehot      (onehot built by DVE is_equal)
GRU is computed feature-major (gates on partitions) so all biases ride free on
ACT/STT per-partition operands.
"""
import sys
from contextlib import ExitStack

import numpy as np

sys.path.insert(0, "/opt/trn_rl_repo")
import ml_dtypes  # noqa: E402

BF16 = ml_dtypes.bfloat16

E = 200000
H = 256
C = 8
DPC = E // C          # 25000 destinations per core
BLK = 128             # destinations per block
NBLK = (DPC + BLK - 1) // BLK   # 196
ROWS = NBLK * BLK     # 25088
SBW = 2               # blocks per GRU superblock (256 rows)
NSB = NBLK // SBW     # 98
SBN = SBW * BLK       # 256 rows per superblock
TILE_E = 128
PADREL = 255
G_SUP = 8             # msg tiles per ef_g supertile (1024 edge slots)
R_SUP = 8             # GRU superblocks per ef_r supertile (2048 rows)

_LAST_RESULTS = None  # stashed BassKernelResults for test harness use


def _build_schedule(dst):
    """Shared cross-core tile schedule. Returns (a_list, per-core packs)."""
    order = np.argsort(dst, kind="stable")
    core_of = dst[order] // DPC
    per_core = []
    for c in range(C):
        sel = order[core_of == c]
        ld = (dst[sel] - c * DPC).astype(np.int64)
        per_core.append((sel, ld, ld // BLK))

    ptr = [0] * C
    ncnt = [len(p[0]) for p in per_core]
    a_list = []
    tiles_eid = [[] for _ in range(C)]
    tiles_rel = [[] for _ in range(C)]
    INF = 1 << 60
    while True:
        nbs = [per_core[c][2][ptr[c]] if ptr[c] < ncnt[c] else INF
               for c in range(C)]
        a_k = min(nbs)
        if a_k == INF:
            break
        a_list.append(int(a_k))
        for c in range(C):
            sel, ld, blk = per_core[c]
            p = ptr[c]
            eids = np.full(TILE_E, -1, np.int64)
            rels = np.full(TILE_E, PADREL, np.int16)
            t = 0
            while p < ncnt[c] and t < TILE_E and blk[p] <= a_k + 1:
                eids[t] = sel[p]
                rels[t] = ld[p] - BLK * a_k
                t += 1
                p += 1
            ptr[c] = p
            tiles_eid[c].append(eids)
            tiles_rel[c].append(rels)
    T = len(a_list)
    Tpad = ((T + G_SUP - 1) // G_SUP) * G_SUP
    for _ in range(Tpad - T):
        a_list.append(a_list[-1] if a_list else 0)
        for c in range(C):
            tiles_eid[c].append(np.full(TILE_E, -1, np.int64))
            tiles_rel[c].append(np.full(TILE_E, PADREL, np.int16))
    packs = [(np.stack(tiles_eid[c]), np.stack(tiles_rel[c])) for c in range(C)]
    return np.asarray(a_list, np.int64), packs


def _coverage(a_list):
    nb_tot = NBLK + 1
    first = np.full(nb_tot, -1, np.int64)
    last = np.full(nb_tot, -1, np.int64)
    for k, a in enumerate(a_list):
        for b in (a, a + 1):
            if b < nb_tot:
                if first[b] < 0:
                    first[b] = k
                last[b] = k
    return first, last


def _host_pack(ef, dst):
    a_list, packs = _build_schedule(dst)
    T = len(a_list)
    ins = []
    for c in range(C):
        eid, rel = packs[c]
        valid = eid.reshape(-1) >= 0
        g = np.zeros((T * TILE_E, H), np.float32)
        g[valid] = ef[eid.reshape(-1)[valid]]
        ef_g = np.zeros((H + 1, T * TILE_E), BF16)
        ef_g[:H] = np.ascontiguousarray(g.T).astype(BF16)
        ef_g[H] = valid.astype(BF16)
        lo = c * DPC
        n_real = min(DPC, E - lo)
        ef_r = np.zeros((H, ROWS), BF16)
        ef_r[:, :n_real] = np.ascontiguousarray(ef[lo:lo + n_real].T).astype(BF16)
        ins.append(dict(
            ef_g=np.ascontiguousarray(ef_g),
            ic=np.ascontiguousarray(np.concatenate(
                [np.broadcast_to(np.arange(256, dtype=np.float32), (128, 256)),
                 rel.T.astype(np.float32)], axis=1)),
            ef_r=np.ascontiguousarray(ef_r),
        ))
    return a_list, ins


def _build_nc(T, a_list, first, last, reps=1, loop_n=0):
    import concourse.tile as tile
    from concourse import bacc, mybir

    dt = mybir.dt
    F = mybir.ActivationFunctionType
    OP = mybir.AluOpType

    nc = bacc.Bacc()
    ef_g = nc.dram_tensor("ef_g", [H + 1, T * TILE_E], dt.bfloat16,
                          kind="ExternalInput")
    ic_d = nc.dram_tensor("ic", [TILE_E, 256 + T], dt.float32,
                          kind="ExternalInput")
    ef_r = nc.dram_tensor("ef_r", [H, ROWS], dt.bfloat16, kind="ExternalInput")
    wmsgT = nc.dram_tensor("wmsgT", [H, H], dt.bfloat16, kind="ExternalInput")
    bmsg = nc.dram_tensor("bmsg", [1, H], dt.bfloat16, kind="ExternalInput")
    wihT = nc.dram_tensor("wihT", [H, 3 * H], dt.bfloat16, kind="ExternalInput")
    whhT = nc.dram_tensor("whhT", [H, 3 * H], dt.bfloat16, kind="ExternalInput")
    brz_d = nc.dram_tensor("brz", [128, 4], dt.float32, kind="ExternalInput")
    bin_d = nc.dram_tensor("bin", [128, 2], dt.float32, kind="ExternalInput")
    bhn_d = nc.dram_tensor("bhn", [128, 2], dt.float32, kind="ExternalInput")
    outT = nc.dram_tensor("outT", [H, ROWS], dt.float32, kind="ExternalOutput")

    NSB_TOT = (NBLK + 1 + SBW - 1) // SBW  # incl. phantom overhang block

    with ExitStack() as ctx:
        tc = ctx.enter_context(tile.TileContext(nc))
        consts = ctx.enter_context(tc.tile_pool(name="consts", bufs=1))
        stg = ctx.enter_context(tc.tile_pool(name="stg", bufs=3))
        rtg = ctx.enter_context(tc.tile_pool(name="rtg", bufs=2))
        msgp = ctx.enter_context(tc.tile_pool(name="msgp", bufs=3))
        gates = ctx.enter_context(tc.tile_pool(name="gates", bufs=3))
        outp = ctx.enter_context(tc.tile_pool(name="outp", bufs=3))
        ps_msg = ctx.enter_context(tc.tile_pool(name="ps_msg", bufs=2,
                                                space="PSUM"))
        ps_agg = ctx.enter_context(tc.tile_pool(name="ps_agg", bufs=2,
                                                space="PSUM"))
        ps_gru = ctx.enter_context(tc.tile_pool(name="ps_gru", bufs=1,
                                                space="PSUM"))

        # ---- constants ----
        ic_sb = consts.tile([128, 256 + T], dt.float32)
        nc.sync.dma_start(out=ic_sb, in_=ic_d[:, :])
        iota_sb = ic_sb[:, 0:256]
        rel_sb = ic_sb[:, 256:256 + T]
        wmsg_sb = consts.tile([128, 2, H], dt.bfloat16)
        nc.sync.dma_start(out=wmsg_sb,
                          in_=wmsgT[:, :].rearrange("(c p) h -> p c h", p=128))
        bmsg_sb = consts.tile([1, H], dt.bfloat16)
        nc.sync.dma_start(out=bmsg_sb, in_=bmsg[:, :])
        wih_sb = consts.tile([128, 2, 3 * H], dt.bfloat16)
        nc.sync.dma_start(out=wih_sb,
                          in_=wihT[:, :].rearrange("(c p) g -> p c g", p=128))
        whh_sb = consts.tile([128, 2, 3 * H], dt.bfloat16)
        nc.sync.dma_start(out=whh_sb,
                          in_=whhT[:, :].rearrange("(c p) g -> p c g", p=128))
        brz_sb = consts.tile([128, 4], dt.float32)
        nc.sync.dma_start(out=brz_sb, in_=brz_d[:, :])
        bin_sb = consts.tile([128, 2], dt.float32)
        nc.sync.dma_start(out=bin_sb, in_=bin_d[:, :])
        bhn_sb = consts.tile([128, 2], dt.float32)
        nc.sync.dma_start(out=bhn_sb, in_=bhn_d[:, :])
        zrow = consts.tile([1, 2 * SBN], dt.bfloat16)
        nc.vector.memset(zrow, 0.0)

        state = {}

        def reset_state():
            state.clear()
            state.update(cur_g={}, cur_r={}, aggps={})


        reset_state()

        def get_aggps(sb):
            aggps = state["aggps"]
            if sb not in aggps:
                t = ps_agg.tile([128, 2 * SBN], dt.float32, tag="aggps")
                aggps[sb] = t
                # One start=True matmul covering the whole bank: clears
                # has_written for the bank and writes zeros, so every
                # subsequent scatter matmul can accumulate with start=False.
                # (start=True clears has_written for the WHOLE bank; with 4
                # interleaved accumulation regions per bank, per-region
                # start flags corrupt sibling regions.)
                nc.tensor.matmul(t[:, :], zrow[0:1, 0:128], zrow[0:1, :],
                                 start=True, stop=False, skip_group_check=True)
            return aggps[sb]

        def emit_tile(k):
            cur_g = state["cur_g"]
            st, off = k // G_SUP, (k % G_SUP) * TILE_E
            if st not in cur_g:
                g0 = stg.tile([128, G_SUP * TILE_E], dt.bfloat16, tag="g0")
                g1 = stg.tile([128, G_SUP * TILE_E], dt.bfloat16, tag="g1")
                vl = stg.tile([1, G_SUP * TILE_E], dt.bfloat16, tag="vl")
                lo, hi = st * G_SUP * TILE_E, (st + 1) * G_SUP * TILE_E
                nc.sync.dma_start(out=g0, in_=ef_g[0:128, lo:hi])
                nc.sync.dma_start(out=g1, in_=ef_g[128:256, lo:hi])
                nc.sync.dma_start(out=vl, in_=ef_g[256:257, lo:hi])
                cur_g.clear()
                cur_g[st] = (g0, g1, vl)
            g0, g1, vl = cur_g[st]
            sl = slice(off, off + TILE_E)

            mp = ps_msg.tile([128, H], dt.float32, tag="msgps")
            nc.tensor.matmul(mp, g0[:, sl], wmsg_sb[:, 0, :],
                             start=True, stop=False)
            nc.tensor.matmul(mp, g1[:, sl], wmsg_sb[:, 1, :],
                             start=False, stop=False)
            nc.tensor.matmul(mp, vl[:, sl], bmsg_sb[0:1, :],
                             start=False, stop=True)
            msg = msgp.tile([128, H], dt.bfloat16, tag="msg")
            nc.scalar.activation(msg, mp, F.Relu)

            oh = msgp.tile([128, 256], dt.bfloat16, tag="oh")
            nc.vector.tensor_scalar(oh, iota_sb, rel_sb[:, k:k + 1], None,
                                    OP.is_equal)

            a = int(a_list[k])
            for half in (0, 1):
                b = a + half
                t = get_aggps(b // SBW)
                j = b % SBW
                for cc in range(2):
                    nc.tensor.matmul(
                        t[:, cc * SBN + j * BLK: cc * SBN + (j + 1) * BLK],
                        msg[:, cc * 128:(cc + 1) * 128],
                        oh[:, half * 128:(half + 1) * 128],
                        start=False, stop=(k == last[b]),
                        skip_group_check=True)

        def emit_gru(s):
            cur_r = state["cur_r"]
            rst, roff = s // R_SUP, (s % R_SUP) * SBN
            if rst not in cur_r:
                r0 = rtg.tile([128, R_SUP * SBN], dt.bfloat16, tag="r0")
                r1 = rtg.tile([128, R_SUP * SBN], dt.bfloat16, tag="r1")
                lo = rst * R_SUP * SBN
                hi = min((rst + 1) * R_SUP * SBN, ROWS)
                nc.sync.dma_start(out=r0[:, :hi - lo], in_=ef_r[0:128, lo:hi])
                nc.sync.dma_start(out=r1[:, :hi - lo], in_=ef_r[128:256, lo:hi])
                cur_r.clear()
                cur_r[rst] = (r0, r1)
            efr = cur_r[rst]

            t = get_aggps(s)
            aggT = []
            for cc in range(2):
                at = gates.tile([128, SBN], dt.bfloat16, tag=f"aggT{cc}")
                nc.vector.tensor_copy(at, t[:, cc * SBN:(cc + 1) * SBN])
                aggT.append(at)

            rz = ps_gru.tile([128, 4 * SBN], dt.float32, tag="rz")
            nh = ps_gru.tile([128, 4 * SBN], dt.float32, tag="nh")
            for g in range(6):
                for src in (0, 1):      # 0 = gi (agg), 1 = gh (ef_r)
                    w = wih_sb if src == 0 else whh_sb
                    for kk in range(2):
                        rhs = (aggT[kk] if src == 0
                               else efr[kk][:, roff:roff + SBN])
                        if g < 4:
                            out = rz[:, g * SBN:(g + 1) * SBN]
                            start = (src == 0 and kk == 0)
                            stop = (src == 1 and kk == 1)
                        else:
                            hs = (g - 4) if src == 0 else (2 + g - 4)
                            out = nh[:, hs * SBN:(hs + 1) * SBN]
                            start, stop = (kk == 0), (kk == 1)
                        nc.tensor.matmul(out, w[:, kk, g * 128:(g + 1) * 128],
                                         rhs, start=start, stop=stop)

            for cc in range(2):
                r_t = gates.tile([128, SBN], dt.bfloat16, tag="r_t")
                nc.scalar.activation(r_t, rz[:, cc * SBN:(cc + 1) * SBN],
                                     F.Sigmoid, bias=brz_sb[:, cc:cc + 1])
                z_t = gates.tile([128, SBN], dt.bfloat16, tag="z_t")
                nc.scalar.activation(z_t, rz[:, (2 + cc) * SBN:(3 + cc) * SBN],
                                     F.Sigmoid, bias=brz_sb[:, 2 + cc:3 + cc])
                hn_t = gates.tile([128, SBN], dt.bfloat16, tag="hn_t")
                nc.scalar.copy(hn_t, nh[:, (2 + cc) * SBN:(3 + cc) * SBN])
                m1 = gates.tile([128, SBN], dt.bfloat16, tag="m1")
                nc.vector.scalar_tensor_tensor(
                    m1, hn_t, bhn_sb[:, cc:cc + 1], r_t, OP.add, OP.mult)
                q_t = gates.tile([128, SBN], dt.bfloat16, tag="q_t")
                nc.vector.tensor_add(q_t, m1, nh[:, cc * SBN:(cc + 1) * SBN])
                n_t = gates.tile([128, SBN], dt.bfloat16, tag="n_t")
                nc.scalar.activation(n_t, q_t, F.Tanh,
                                     bias=bin_sb[:, cc:cc + 1])
                d_t = gates.tile([128, SBN], dt.bfloat16, tag="d_t")
                nc.gpsimd.tensor_sub(d_t, efr[cc][:, roff:roff + SBN], n_t)
                v_t = gates.tile([128, SBN], dt.bfloat16, tag="v_t")
                nc.gpsimd.tensor_mul(v_t, z_t, d_t)
                o_t = outp.tile([128, SBN], dt.float32, tag="o_t")
                nc.gpsimd.tensor_add(o_t, n_t, v_t)
                nc.sync.dma_start(
                    out=outT[cc * 128:(cc + 1) * 128,
                             s * SBN:(s + 1) * SBN],
                    in_=o_t)

        def emit_body():
            for _rep in range(reps):
                reset_state()
                kptr = 0
                for s in range(NSB):
                    while kptr < T and a_list[kptr] <= s * SBW + (SBW - 1):
                        emit_tile(kptr)
                        kptr += 1
                    emit_gru(s)
                while kptr < T:
                    emit_tile(kptr)
                    kptr += 1

        if loop_n > 1:
            with tc.For_i(0, loop_n, 1):
                emit_body()
        else:
            emit_body()
        del NSB_TOT
    if not nc.is_finalized():
        nc.finalize()
    return nc


def _pack_weights(W_msg_w, W_msg_b, gru_w_ih, gru_w_hh, gru_b_ih, gru_b_hh):
    wmsgT = np.ascontiguousarray(W_msg_w.astype(np.float32).T).astype(BF16)
    bmsg = W_msg_b.astype(np.float32).reshape(1, H).astype(BF16)
    wihT = np.ascontiguousarray(gru_w_ih.astype(np.float32).T).astype(BF16)
    whhT = np.ascontiguousarray(gru_w_hh.astype(np.float32).T).astype(BF16)
    bi = gru_b_ih.astype(np.float32)
    bh = gru_b_hh.astype(np.float32)
    comb = bi + bh
    brz = np.stack([comb[0:128], comb[128:256], comb[256:384], comb[384:512]],
                   axis=1)
    bin_ = np.stack([bi[512:640], bi[640:768]], axis=1)
    bhn = np.stack([bh[512:640], bh[640:768]], axis=1)
    iota = np.broadcast_to(np.arange(256, dtype=np.int16), (128, 256))
    return dict(
        wmsgT=wmsgT, bmsg=np.ascontiguousarray(bmsg),
        wihT=wihT, whhT=whhT,
        brz=np.ascontiguousarray(brz), bin=np.ascontiguousarray(bin_),
        bhn=np.ascontiguousarray(bhn),
        iota=np.ascontiguousarray(iota),
    )


def _run_pjrt(nc, in_maps, n_cores, time_runs=0, return_fn=False):
    """Execute the Bass program on n_cores NeuronCores via the axon PJRT
    path (mirrors bass2jax.run_bass_via_pjrt, minus output donation so the
    jitted callable can be re-invoked for timing)."""
    import jax
    import numpy as _np
    from jax.sharding import Mesh, NamedSharding, PartitionSpec
    from jax.experimental.shard_map import shard_map
    from concourse import bass2jax as B
    from concourse import mybir

    B.install_neuronx_cc_hook()

    partition_name = (nc.partition_id_tensor.name
                      if nc.partition_id_tensor is not None else None)
    in_names, out_names, out_avals, zero_outs = [], [], [], []
    for alloc in nc.m.functions[0].allocations:
        if not isinstance(alloc, mybir.MemoryLocationSet):
            continue
        name = alloc.memorylocations[0].name
        if alloc.kind == "ExternalInput":
            if name != partition_name:
                in_names.append(name)
        elif alloc.kind == "ExternalOutput":
            shape = tuple(alloc.tensor_shape)
            dtype = mybir.dt.np(alloc.dtype)
            out_names.append(name)
            out_avals.append(jax.core.ShapedArray(shape, dtype))
            zero_outs.append(_np.zeros(shape, dtype))
    n_params = len(in_names)
    all_names = in_names + out_names
    if partition_name is not None:
        all_names = all_names + [partition_name]

    def _body(*args):
        operands = list(args)
        if partition_name is not None:
            operands.append(B.partition_id_tensor())
        outs = B._bass_exec_p.bind(
            *operands,
            out_avals=tuple(out_avals),
            in_names=tuple(all_names),
            out_names=tuple(out_names),
            lowering_input_output_aliases=(),
            sim_require_finite=True,
            sim_require_nnan=True,
            nc=nc,
        )
        return tuple(outs)

    devices = jax.devices()[:n_cores]
    mesh = Mesh(_np.asarray(devices), ("core",))
    spec = NamedSharding(mesh, PartitionSpec("core"))
    nin = n_params + len(zero_outs)
    fn = jax.jit(
        shard_map(_body, mesh=mesh,
                  in_specs=(PartitionSpec("core"),) * nin,
                  out_specs=(PartitionSpec("core"),) * len(out_names),
                  check_rep=False),
        keep_unused=True,
    )
    concat = [
        _np.concatenate([_np.asarray(in_maps[c][nm]) for c in range(n_cores)],
                        axis=0)
        for nm in in_names
    ] + [
        _np.zeros((n_cores * z.shape[0], *z.shape[1:]), z.dtype)
        for z in zero_outs
    ]
    dev_args = [jax.device_put(a, spec) for a in concat]
    outs = fn(*dev_args)
    jax.block_until_ready(outs)
    if return_fn:
        return fn, dev_args, outs, out_names, out_avals

    times = []
    for _ in range(time_runs):
        import time as _time
        t0 = _time.perf_counter()
        o = fn(*dev_args)
        jax.block_until_ready(o)
        times.append(_time.perf_counter() - t0)

    results = [
        {nm: _np.asarray(outs[i]).reshape(n_cores, *out_avals[i].shape)[c]
         for i, nm in enumerate(out_names)}
        for c in range(n_cores)
    ]
    return results, times


def kernel(edge_features, edge_index, W_msg_w, W_msg_b,
           gru_w_ih, gru_w_hh, gru_b_ih, gru_b_hh):
    global _LAST_RESULTS
    import os

    ef = np.asarray(edge_features, np.float32)
    dst = np.asarray(edge_index)[1].astype(np.int64)

    a_list, per_core = _host_pack(ef, dst)
    first, last = _coverage(a_list)
    T = len(a_list)
    reps = int(os.environ.get("DMPNN_REPS", "1"))
    nc = _build_nc(T, a_list, first, last, reps=reps)

    wpack = _pack_weights(np.asarray(W_msg_w), np.asarray(W_msg_b),
                          np.asarray(gru_w_ih), np.asarray(gru_w_hh),
                          np.asarray(gru_b_ih), np.asarray(gru_b_hh))
    in_maps = [{**per_core[c], **wpack} for c in range(C)]

    time_runs = int(os.environ.get("DMPNN_TIME_RUNS", "0"))
    results, times = _run_pjrt(nc, in_maps, C, time_runs=time_runs)
    _LAST_RESULTS = dict(times=times)

    full = np.empty((E, H), np.float32)
    for c in range(C):
        o = np.asarray(results[c]["outT"])  # [H, ROWS] f32
        full[c * DPC:(c + 1) * DPC] = o[:, :DPC].T
    return full


# revision 16
# speedup vs baseline: 141.3385x; 141.3385x over previous
"""DMPNN layer (message passing + scatter-add + GRU) on 8 Trainium2 cores.

Strategy: sort edges by destination on the host, partition destinations into
8 contiguous ranges (one per core). Each core computes messages for the edges
that TARGET its destination range, scatter-adds them locally into its
destination rows (no cross-core traffic), then runs the GRU on its row range.

On-device scatter-add: edges are packed (host-side) into 128-edge tiles, each
tile constrained to a 256-destination window [128*a_k, 128*a_k+256). The
window schedule {a_k} is shared by all 8 cores (SPMD: one program) and is
computed from the union of all cores' needs; slack is absorbed by zero-message
pad edges. Scatter = one-hot matmuls on the TensorEngine:
    aggT[:, win] += msg_chunk^T @ onehot      (onehot built by DVE is_equal)
GRU is computed feature-major (gates on partitions) so all biases ride free on
ACT/STT per-partition operands.
"""
import sys
from contextlib import ExitStack

import numpy as np

sys.path.insert(0, "/opt/trn_rl_repo")
import ml_dtypes  # noqa: E402

BF16 = ml_dtypes.bfloat16

E = 200000
H = 256
C = 8
DPC = E // C          # 25000 destinations per core
BLK = 128             # destinations per block
NBLK = (DPC + BLK - 1) // BLK   # 196
ROWS = NBLK * BLK     # 25088
SBW = 2               # blocks per GRU superblock (256 rows)
NSB = NBLK // SBW     # 98
SBN = SBW * BLK       # 256 rows per superblock
TILE_E = 128
PADREL = 255
G_SUP = 8             # msg tiles per ef_g supertile (1024 edge slots)
R_SUP = 8             # GRU superblocks per ef_r supertile (2048 rows)

_LAST_RESULTS = None  # stashed BassKernelResults for test harness use


def _build_schedule(dst):
    """Shared cross-core tile schedule. Returns (a_list, per-core packs)."""
    order = np.argsort(dst, kind="stable")
    core_of = dst[order] // DPC
    per_core = []
    for c in range(C):
        sel = order[core_of == c]
        ld = (dst[sel] - c * DPC).astype(np.int64)
        per_core.append((sel, ld, ld // BLK))

    ptr = [0] * C
    ncnt = [len(p[0]) for p in per_core]
    a_list = []
    tiles_eid = [[] for _ in range(C)]
    tiles_rel = [[] for _ in range(C)]
    INF = 1 << 60
    while True:
        nbs = [per_core[c][2][ptr[c]] if ptr[c] < ncnt[c] else INF
               for c in range(C)]
        a_k = min(nbs)
        if a_k == INF:
            break
        a_list.append(int(a_k))
        for c in range(C):
            sel, ld, blk = per_core[c]
            p = ptr[c]
            eids = np.full(TILE_E, -1, np.int64)
            rels = np.full(TILE_E, PADREL, np.int16)
            t = 0
            while p < ncnt[c] and t < TILE_E and blk[p] <= a_k + 1:
                eids[t] = sel[p]
                rels[t] = ld[p] - BLK * a_k
                t += 1
                p += 1
            ptr[c] = p
            tiles_eid[c].append(eids)
            tiles_rel[c].append(rels)
    T = len(a_list)
    Tpad = ((T + G_SUP - 1) // G_SUP) * G_SUP
    for _ in range(Tpad - T):
        a_list.append(a_list[-1] if a_list else 0)
        for c in range(C):
            tiles_eid[c].append(np.full(TILE_E, -1, np.int64))
            tiles_rel[c].append(np.full(TILE_E, PADREL, np.int16))
    packs = [(np.stack(tiles_eid[c]), np.stack(tiles_rel[c])) for c in range(C)]
    return np.asarray(a_list, np.int64), packs


def _coverage(a_list):
    nb_tot = NBLK + 1
    first = np.full(nb_tot, -1, np.int64)
    last = np.full(nb_tot, -1, np.int64)
    for k, a in enumerate(a_list):
        for b in (a, a + 1):
            if b < nb_tot:
                if first[b] < 0:
                    first[b] = k
                last[b] = k
    return first, last


def _host_pack(ef, dst):
    a_list, packs = _build_schedule(dst)
    T = len(a_list)
    ins = []
    for c in range(C):
        eid, rel = packs[c]
        valid = eid.reshape(-1) >= 0
        g = np.zeros((T * TILE_E, H), np.float32)
        g[valid] = ef[eid.reshape(-1)[valid]]
        ef_g = np.zeros((H + 1, T * TILE_E), BF16)
        ef_g[:H] = np.ascontiguousarray(g.T).astype(BF16)
        ef_g[H] = valid.astype(BF16)
        lo = c * DPC
        n_real = min(DPC, E - lo)
        ef_r = np.zeros((H, ROWS), BF16)
        ef_r[:, :n_real] = np.ascontiguousarray(ef[lo:lo + n_real].T).astype(BF16)
        ins.append(dict(
            ef_g=np.ascontiguousarray(ef_g),
            ic=np.ascontiguousarray(np.concatenate(
                [np.broadcast_to(np.arange(256, dtype=np.float32), (128, 256)),
                 rel.T.astype(np.float32)], axis=1)),
            ef_r=np.ascontiguousarray(ef_r),
        ))
    return a_list, ins


def _build_nc(T, a_list, first, last, reps=1, loop_n=0):
    import concourse.tile as tile
    from concourse import bacc, mybir

    dt = mybir.dt
    F = mybir.ActivationFunctionType
    OP = mybir.AluOpType

    nc = bacc.Bacc()
    ef_g = nc.dram_tensor("ef_g", [H + 1, T * TILE_E], dt.bfloat16,
                          kind="ExternalInput")
    ic_d = nc.dram_tensor("ic", [TILE_E, 256 + T], dt.float32,
                          kind="ExternalInput")
    ef_r = nc.dram_tensor("ef_r", [H, ROWS], dt.bfloat16, kind="ExternalInput")
    wmsgT = nc.dram_tensor("wmsgT", [H, H], dt.bfloat16, kind="ExternalInput")
    bmsg = nc.dram_tensor("bmsg", [1, H], dt.bfloat16, kind="ExternalInput")
    wihT = nc.dram_tensor("wihT", [H, 3 * H], dt.bfloat16, kind="ExternalInput")
    whhT = nc.dram_tensor("whhT", [H, 3 * H], dt.bfloat16, kind="ExternalInput")
    brz_d = nc.dram_tensor("brz", [128, 4], dt.float32, kind="ExternalInput")
    bin_d = nc.dram_tensor("bin", [128, 2], dt.float32, kind="ExternalInput")
    bhn_d = nc.dram_tensor("bhn", [128, 2], dt.float32, kind="ExternalInput")
    outT = nc.dram_tensor("outT", [H, ROWS], dt.float32, kind="ExternalOutput")

    NSB_TOT = (NBLK + 1 + SBW - 1) // SBW  # incl. phantom overhang block

    with ExitStack() as ctx:
        tc = ctx.enter_context(tile.TileContext(nc))
        consts = ctx.enter_context(tc.tile_pool(name="consts", bufs=1))
        stg = ctx.enter_context(tc.tile_pool(name="stg", bufs=4))
        rtg = ctx.enter_context(tc.tile_pool(name="rtg", bufs=3))
        msgp = ctx.enter_context(tc.tile_pool(name="msgp", bufs=4))
        gates = ctx.enter_context(tc.tile_pool(name="gates", bufs=4))
        outp = ctx.enter_context(tc.tile_pool(name="outp", bufs=3))
        ps_msg = ctx.enter_context(tc.tile_pool(name="ps_msg", bufs=2,
                                                space="PSUM"))
        ps_agg = ctx.enter_context(tc.tile_pool(name="ps_agg", bufs=2,
                                                space="PSUM"))
        ps_gru = ctx.enter_context(tc.tile_pool(name="ps_gru", bufs=1,
                                                space="PSUM"))

        # ---- constants ----
        ic_sb = consts.tile([128, 256 + T], dt.float32)
        nc.sync.dma_start(out=ic_sb, in_=ic_d[:, :])
        iota_sb = ic_sb[:, 0:256]
        rel_sb = ic_sb[:, 256:256 + T]
        wmsg_sb = consts.tile([128, 2, H], dt.bfloat16)
        nc.sync.dma_start(out=wmsg_sb,
                          in_=wmsgT[:, :].rearrange("(c p) h -> p c h", p=128))
        bmsg_sb = consts.tile([1, H], dt.bfloat16)
        nc.sync.dma_start(out=bmsg_sb, in_=bmsg[:, :])
        wih_sb = consts.tile([128, 2, 3 * H], dt.bfloat16)
        nc.sync.dma_start(out=wih_sb,
                          in_=wihT[:, :].rearrange("(c p) g -> p c g", p=128))
        whh_sb = consts.tile([128, 2, 3 * H], dt.bfloat16)
        nc.sync.dma_start(out=whh_sb,
                          in_=whhT[:, :].rearrange("(c p) g -> p c g", p=128))
        brz_sb = consts.tile([128, 4], dt.float32)
        nc.sync.dma_start(out=brz_sb, in_=brz_d[:, :])
        bin_sb = consts.tile([128, 2], dt.float32)
        nc.sync.dma_start(out=bin_sb, in_=bin_d[:, :])
        bhn_sb = consts.tile([128, 2], dt.float32)
        nc.sync.dma_start(out=bhn_sb, in_=bhn_d[:, :])
        zrow = consts.tile([1, 2 * SBN], dt.bfloat16)
        nc.vector.memset(zrow, 0.0)

        state = {}

        def reset_state():
            state.clear()
            state.update(cur_g={}, cur_r={}, aggps={})


        reset_state()

        def get_aggps(sb):
            aggps = state["aggps"]
            if sb not in aggps:
                t = ps_agg.tile([128, 2 * SBN], dt.float32, tag="aggps")
                aggps[sb] = t
                # One start=True matmul covering the whole bank: clears
                # has_written for the bank and writes zeros, so every
                # subsequent scatter matmul can accumulate with start=False.
                # (start=True clears has_written for the WHOLE bank; with 4
                # interleaved accumulation regions per bank, per-region
                # start flags corrupt sibling regions.)
                nc.tensor.matmul(t[:, :], zrow[0:1, 0:128], zrow[0:1, :],
                                 start=True, stop=False, skip_group_check=True)
            return aggps[sb]

        def emit_tile(k):
            cur_g = state["cur_g"]
            st, off = k // G_SUP, (k % G_SUP) * TILE_E
            if st not in cur_g:
                g0 = stg.tile([128, G_SUP * TILE_E], dt.bfloat16, tag="g0")
                g1 = stg.tile([128, G_SUP * TILE_E], dt.bfloat16, tag="g1")
                vl = stg.tile([1, G_SUP * TILE_E], dt.bfloat16, tag="vl")
                lo, hi = st * G_SUP * TILE_E, (st + 1) * G_SUP * TILE_E
                nc.sync.dma_start(out=g0, in_=ef_g[0:128, lo:hi])
                nc.sync.dma_start(out=g1, in_=ef_g[128:256, lo:hi])
                nc.sync.dma_start(out=vl, in_=ef_g[256:257, lo:hi])
                cur_g.clear()
                cur_g[st] = (g0, g1, vl)
            g0, g1, vl = cur_g[st]
            sl = slice(off, off + TILE_E)

            mp = ps_msg.tile([128, H], dt.float32, tag="msgps")
            nc.tensor.matmul(mp, g0[:, sl], wmsg_sb[:, 0, :],
                             start=True, stop=False)
            nc.tensor.matmul(mp, g1[:, sl], wmsg_sb[:, 1, :],
                             start=False, stop=False)
            nc.tensor.matmul(mp, vl[:, sl], bmsg_sb[0:1, :],
                             start=False, stop=True)
            msg = msgp.tile([128, H], dt.bfloat16, tag="msg")
            nc.scalar.activation(msg, mp, F.Relu)

            oh = msgp.tile([128, 256], dt.bfloat16, tag="oh")
            nc.vector.tensor_scalar(oh, iota_sb, rel_sb[:, k:k + 1], None,
                                    OP.is_equal)

            a = int(a_list[k])
            for half in (0, 1):
                b = a + half
                t = get_aggps(b // SBW)
                j = b % SBW
                for cc in range(2):
                    nc.tensor.matmul(
                        t[:, cc * SBN + j * BLK: cc * SBN + (j + 1) * BLK],
                        msg[:, cc * 128:(cc + 1) * 128],
                        oh[:, half * 128:(half + 1) * 128],
                        start=False, stop=(k == last[b]),
                        skip_group_check=True)

        def emit_gru(s):
            cur_r = state["cur_r"]
            rst, roff = s // R_SUP, (s % R_SUP) * SBN
            if rst not in cur_r:
                r0 = rtg.tile([128, R_SUP * SBN], dt.bfloat16, tag="r0")
                r1 = rtg.tile([128, R_SUP * SBN], dt.bfloat16, tag="r1")
                lo = rst * R_SUP * SBN
                hi = min((rst + 1) * R_SUP * SBN, ROWS)
                nc.sync.dma_start(out=r0[:, :hi - lo], in_=ef_r[0:128, lo:hi])
                nc.sync.dma_start(out=r1[:, :hi - lo], in_=ef_r[128:256, lo:hi])
                cur_r.clear()
                cur_r[rst] = (r0, r1)
            efr = cur_r[rst]

            t = get_aggps(s)
            aggT = []
            for cc in range(2):
                at = gates.tile([128, SBN], dt.bfloat16, tag=f"aggT{cc}")
                nc.vector.tensor_copy(at, t[:, cc * SBN:(cc + 1) * SBN])
                aggT.append(at)

            rz = ps_gru.tile([128, 4 * SBN], dt.float32, tag="rz")
            nh = ps_gru.tile([128, 4 * SBN], dt.float32, tag="nh")
            for g in range(6):
                for src in (0, 1):      # 0 = gi (agg), 1 = gh (ef_r)
                    w = wih_sb if src == 0 else whh_sb
                    for kk in range(2):
                        rhs = (aggT[kk] if src == 0
                               else efr[kk][:, roff:roff + SBN])
                        if g < 4:
                            out = rz[:, g * SBN:(g + 1) * SBN]
                            start = (src == 0 and kk == 0)
                            stop = (src == 1 and kk == 1)
                        else:
                            hs = (g - 4) if src == 0 else (2 + g - 4)
                            out = nh[:, hs * SBN:(hs + 1) * SBN]
                            start, stop = (kk == 0), (kk == 1)
                        nc.tensor.matmul(out, w[:, kk, g * 128:(g + 1) * 128],
                                         rhs, start=start, stop=stop)

            for cc in range(2):
                r_t = gates.tile([128, SBN], dt.bfloat16, tag="r_t")
                nc.scalar.activation(r_t, rz[:, cc * SBN:(cc + 1) * SBN],
                                     F.Sigmoid, bias=brz_sb[:, cc:cc + 1])
                z_t = gates.tile([128, SBN], dt.bfloat16, tag="z_t")
                nc.scalar.activation(z_t, rz[:, (2 + cc) * SBN:(3 + cc) * SBN],
                                     F.Sigmoid, bias=brz_sb[:, 2 + cc:3 + cc])
                m1 = gates.tile([128, SBN], dt.bfloat16, tag="m1")
                nc.vector.scalar_tensor_tensor(
                    m1, nh[:, (2 + cc) * SBN:(3 + cc) * SBN],
                    bhn_sb[:, cc:cc + 1], r_t, OP.add, OP.mult)
                q_t = gates.tile([128, SBN], dt.bfloat16, tag="q_t")
                nc.vector.tensor_add(q_t, m1, nh[:, cc * SBN:(cc + 1) * SBN])
                n_t = gates.tile([128, SBN], dt.bfloat16, tag="n_t")
                nc.scalar.activation(n_t, q_t, F.Tanh,
                                     bias=bin_sb[:, cc:cc + 1])
                d_t = gates.tile([128, SBN], dt.bfloat16, tag="d_t")
                nc.gpsimd.tensor_sub(d_t, efr[cc][:, roff:roff + SBN], n_t)
                v_t = gates.tile([128, SBN], dt.bfloat16, tag="v_t")
                nc.gpsimd.tensor_mul(v_t, z_t, d_t)
                o_t = outp.tile([128, SBN], dt.float32, tag="o_t")
                nc.gpsimd.tensor_add(o_t, n_t, v_t)
                nc.sync.dma_start(
                    out=outT[cc * 128:(cc + 1) * 128,
                             s * SBN:(s + 1) * SBN],
                    in_=o_t)

        def emit_body():
            for _rep in range(reps):
                reset_state()
                kptr = 0
                for s in range(NSB):
                    while kptr < T and a_list[kptr] <= s * SBW + (SBW - 1):
                        emit_tile(kptr)
                        kptr += 1
                    emit_gru(s)
                while kptr < T:
                    emit_tile(kptr)
                    kptr += 1

        if loop_n > 1:
            with tc.For_i(0, loop_n, 1):
                emit_body()
        else:
            emit_body()
        del NSB_TOT
    if not nc.is_finalized():
        nc.finalize()
    return nc


def _pack_weights(W_msg_w, W_msg_b, gru_w_ih, gru_w_hh, gru_b_ih, gru_b_hh):
    wmsgT = np.ascontiguousarray(W_msg_w.astype(np.float32).T).astype(BF16)
    bmsg = W_msg_b.astype(np.float32).reshape(1, H).astype(BF16)
    wihT = np.ascontiguousarray(gru_w_ih.astype(np.float32).T).astype(BF16)
    whhT = np.ascontiguousarray(gru_w_hh.astype(np.float32).T).astype(BF16)
    bi = gru_b_ih.astype(np.float32)
    bh = gru_b_hh.astype(np.float32)
    comb = bi + bh
    brz = np.stack([comb[0:128], comb[128:256], comb[256:384], comb[384:512]],
                   axis=1)
    bin_ = np.stack([bi[512:640], bi[640:768]], axis=1)
    bhn = np.stack([bh[512:640], bh[640:768]], axis=1)
    iota = np.broadcast_to(np.arange(256, dtype=np.int16), (128, 256))
    return dict(
        wmsgT=wmsgT, bmsg=np.ascontiguousarray(bmsg),
        wihT=wihT, whhT=whhT,
        brz=np.ascontiguousarray(brz), bin=np.ascontiguousarray(bin_),
        bhn=np.ascontiguousarray(bhn),
        iota=np.ascontiguousarray(iota),
    )


def _run_pjrt(nc, in_maps, n_cores, time_runs=0, return_fn=False):
    """Execute the Bass program on n_cores NeuronCores via the axon PJRT
    path (mirrors bass2jax.run_bass_via_pjrt, minus output donation so the
    jitted callable can be re-invoked for timing)."""
    import jax
    import numpy as _np
    from jax.sharding import Mesh, NamedSharding, PartitionSpec
    from jax.experimental.shard_map import shard_map
    from concourse import bass2jax as B
    from concourse import mybir

    B.install_neuronx_cc_hook()

    partition_name = (nc.partition_id_tensor.name
                      if nc.partition_id_tensor is not None else None)
    in_names, out_names, out_avals, zero_outs = [], [], [], []
    for alloc in nc.m.functions[0].allocations:
        if not isinstance(alloc, mybir.MemoryLocationSet):
            continue
        name = alloc.memorylocations[0].name
        if alloc.kind == "ExternalInput":
            if name != partition_name:
                in_names.append(name)
        elif alloc.kind == "ExternalOutput":
            shape = tuple(alloc.tensor_shape)
            dtype = mybir.dt.np(alloc.dtype)
            out_names.append(name)
            out_avals.append(jax.core.ShapedArray(shape, dtype))
            zero_outs.append(_np.zeros(shape, dtype))
    n_params = len(in_names)
    all_names = in_names + out_names
    if partition_name is not None:
        all_names = all_names + [partition_name]

    def _body(*args):
        operands = list(args)
        if partition_name is not None:
            operands.append(B.partition_id_tensor())
        outs = B._bass_exec_p.bind(
            *operands,
            out_avals=tuple(out_avals),
            in_names=tuple(all_names),
            out_names=tuple(out_names),
            lowering_input_output_aliases=(),
            sim_require_finite=True,
            sim_require_nnan=True,
            nc=nc,
        )
        return tuple(outs)

    devices = jax.devices()[:n_cores]
    mesh = Mesh(_np.asarray(devices), ("core",))
    spec = NamedSharding(mesh, PartitionSpec("core"))
    nin = n_params + len(zero_outs)
    fn = jax.jit(
        shard_map(_body, mesh=mesh,
                  in_specs=(PartitionSpec("core"),) * nin,
                  out_specs=(PartitionSpec("core"),) * len(out_names),
                  check_rep=False),
        keep_unused=True,
    )
    concat = [
        _np.concatenate([_np.asarray(in_maps[c][nm]) for c in range(n_cores)],
                        axis=0)
        for nm in in_names
    ] + [
        _np.zeros((n_cores * z.shape[0], *z.shape[1:]), z.dtype)
        for z in zero_outs
    ]
    dev_args = [jax.device_put(a, spec) for a in concat]
    outs = fn(*dev_args)
    jax.block_until_ready(outs)
    if return_fn:
        return fn, dev_args, outs, out_names, out_avals

    times = []
    for _ in range(time_runs):
        import time as _time
        t0 = _time.perf_counter()
        o = fn(*dev_args)
        jax.block_until_ready(o)
        times.append(_time.perf_counter() - t0)

    results = [
        {nm: _np.asarray(outs[i]).reshape(n_cores, *out_avals[i].shape)[c]
         for i, nm in enumerate(out_names)}
        for c in range(n_cores)
    ]
    return results, times


def kernel(edge_features, edge_index, W_msg_w, W_msg_b,
           gru_w_ih, gru_w_hh, gru_b_ih, gru_b_hh):
    global _LAST_RESULTS
    import os

    ef = np.asarray(edge_features, np.float32)
    dst = np.asarray(edge_index)[1].astype(np.int64)

    a_list, per_core = _host_pack(ef, dst)
    first, last = _coverage(a_list)
    T = len(a_list)
    reps = int(os.environ.get("DMPNN_REPS", "1"))
    nc = _build_nc(T, a_list, first, last, reps=reps)

    wpack = _pack_weights(np.asarray(W_msg_w), np.asarray(W_msg_b),
                          np.asarray(gru_w_ih), np.asarray(gru_w_hh),
                          np.asarray(gru_b_ih), np.asarray(gru_b_hh))
    in_maps = [{**per_core[c], **wpack} for c in range(C)]

    time_runs = int(os.environ.get("DMPNN_TIME_RUNS", "0"))
    results, times = _run_pjrt(nc, in_maps, C, time_runs=time_runs)
    _LAST_RESULTS = dict(times=times)

    full = np.empty((E, H), np.float32)
    for c in range(C):
        o = np.asarray(results[c]["outT"])  # [H, ROWS] f32
        full[c * DPC:(c + 1) * DPC] = o[:, :DPC].T
    return full
